# revision 1
# baseline (speedup 1.0000x reference)
"""MHA kernel for trn2: 8 cores = 2 (batch DP) x 4 (head TP, 4 heads/core).

Layout strategy (all device tensors d-major / transposed so no on-device
transposes are ever needed):
  - x^T [C, T] per batch (host-transposed, partition-relayout for 1-desc DMA)
  - Q^T/K^T computed as [d, t] via lhsT=W-slice, rhs=x^T; RoPE via PE
    half-swap permutation matmul + signed sin table
  - V computed as [t, d] via lhsT=x^T tile, rhs=Wv (plus ones column for
    softmax denominators)
  - S^T duos [tk=128, 2*512]; 2-head packing on the PE (K=64, base
    partitions 0/64); causal block skipping; exp per duo; software-pipelined
    AV one duo behind S so PE never waits on ACT
  - AV accumulates O_aug^T [65, tq] per head; row 64 = softmax denom
  - normalize via reciprocal + PE broadcast matmul, per q-chunk
  - AllGather y^T per 512-col chunk (4 CC ops) overlapped with compute of
    the next chunk; projection per chunk one step behind the AG
Host reassembles: concat cout slices, transpose, stack batches.
"""

import sys

sys.path.insert(0, "/opt/trn_rl_repo")

from contextlib import ExitStack  # noqa: E402

import numpy as np  # noqa: E402

import concourse.bacc as bacc  # noqa: E402
import concourse.bass as bass  # noqa: E402
import concourse.tile as tile  # noqa: E402
from concourse import mybir  # noqa: E402
from concourse.bass_utils import run_bass_kernel_spmd  # noqa: E402

B, T, C, H = 2, 2048, 1024, 16
HD, HD2 = 64, 32
NCORES, GROUPS, HPG, NPAIRS = 8, 4, 4, 2
TK, TQ = 128, 512
NQ = T // TQ  # 4 q-chunks
NKT = T // TK  # 16 tk tiles
KT = C // 128  # 8 contraction tiles
DGRP = 256  # head dims per core (4 heads * 64)
NIDX = NPAIRS * NQ  # 8 (p, qi) output blocks

F32 = mybir.dt.float32
F32R = mybir.dt.float32r
AF = mybir.ActivationFunctionType
ALU = mybir.AluOpType
SCALE = 1.0 / 8.0  # 1/sqrt(HD)


def r32(ap):
    return ap.bitcast(F32R)


def build_nc():
    nc = bacc.Bacc(target_bir_lowering=False)

    xr = nc.dram_tensor("xr", [128, KT * T], F32, kind="ExternalInput")
    wqr = nc.dram_tensor("wqr", [128, KT * DGRP], F32, kind="ExternalInput")
    wkr = nc.dram_tensor("wkr", [128, KT * DGRP], F32, kind="ExternalInput")
    wvr = nc.dram_tensor("wvr", [128, KT * DGRP], F32, kind="ExternalInput")
    wor = nc.dram_tensor("wor", [128, KT * DGRP], F32, kind="ExternalInput")
    cos4 = nc.dram_tensor("cos4", [128, T], F32, kind="ExternalInput")
    sin4 = nc.dram_tensor("sin4", [128, T], F32, kind="ExternalInput")
    perm = nc.dram_tensor("perm", [128, 128], F32, kind="ExternalInput")
    tri = nc.dram_tensor("tri", [TK, TK], F32, kind="ExternalInput")
    bq = nc.dram_tensor("bq", [DGRP, 1], F32, kind="ExternalInput")
    bk = nc.dram_tensor("bk", [DGRP, 1], F32, kind="ExternalInput")
    bo = nc.dram_tensor("bo", [DGRP, 1], F32, kind="ExternalInput")
    sel = nc.dram_tensor("sel", [2 * NPAIRS, NPAIRS, 128], F32, kind="ExternalInput")
    out = nc.dram_tensor("out", [DGRP, T], F32, kind="ExternalOutput")

    with tile.TileContext(nc) as tc, ExitStack() as top:
        dram = top.enter_context(tc.tile_pool(name="dram", bufs=1, space="DRAM"))
        y_loc_q = [dram.tile([DGRP, TQ], F32, name=f"ylq{q}") for q in range(NQ)]
        y_all_q = [
            dram.tile([GROUPS * DGRP, TQ], F32, name=f"yaq{q}") for q in range(NQ)
        ]
        consts = top.enter_context(tc.tile_pool(name="consts", bufs=1))
        cos_sb = consts.tile([128, T], F32)
        sin_sb = consts.tile([128, T], F32)
        perm_sb = consts.tile([128, 128], F32)
        tri_sb = consts.tile([TK, TK], F32)
        bq_sb = consts.tile([128, NPAIRS], F32)
        bk_sb = consts.tile([128, NPAIRS], F32)
        bo_sb = consts.tile([128, NPAIRS], F32)
        sel_sb = consts.tile([2 * NPAIRS, NPAIRS, 128], F32)
        bqr = bq.ap().rearrange("(p c) one -> c (p one)", c=128)
        bkr = bk.ap().rearrange("(p c) one -> c (p one)", c=128)
        bor = bo.ap().rearrange("(p c) one -> c (p one)", c=128)

        wo_pool = top.enter_context(tc.tile_pool(name="wo", bufs=1))
        wo_sb = wo_pool.tile([128, KT, DGRP], F32)

        rqk_pool = top.enter_context(tc.tile_pool(name="rqk", bufs=1))
        # RQ/RK per pair: [128, T]; rows = (u1 h0, u2 h0, u1 h1, u2 h1) x 32
        RQ = [rqk_pool.tile([128, T], F32, name=f"RQ{p}") for p in range(NPAIRS)]
        RK = [rqk_pool.tile([128, T], F32, name=f"RK{p}") for p in range(NPAIRS)]
        v_pool = top.enter_context(tc.tile_pool(name="vsb", bufs=1))
        V_sb = v_pool.tile([128, NKT, HPG, HD + 1], F32)

        # ---------------- Phase 1: QKV projection + RoPE ----------------
        with ExitStack() as ph1:
            xt_pool = ph1.enter_context(tc.tile_pool(name="xt", bufs=1))
            xT_sb = xt_pool.tile([128, NQ, KT, TQ], F32)
            wqk_pool = ph1.enter_context(tc.tile_pool(name="wqk", bufs=1))
            wq_sb = wqk_pool.tile([128, KT, DGRP], F32)
            wk_sb = wqk_pool.tile([128, KT, DGRP], F32)
            wv_sb = wqk_pool.tile([128, KT, DGRP], F32)

            # DMA issue order = completion order: feed the PE's phase-1
            # consumption sequence (Q needs wq + x blocks first)
            xrv = xr.ap().rearrange("p (n k t) -> p n k t", n=NQ, k=KT)
            nc.gpsimd.dma_start(out=r32(wq_sb), in_=wqr.ap())
            nc.gpsimd.dma_start(out=r32(xT_sb[:, 0]), in_=xrv[:, 0])
            nc.gpsimd.dma_start(out=r32(perm_sb), in_=perm.ap())
            nc.gpsimd.dma_start(out=bq_sb, in_=bqr)
            nc.gpsimd.dma_start(out=bk_sb, in_=bkr)
            nc.gpsimd.dma_start(out=r32(xT_sb[:, 1]), in_=xrv[:, 1])
            nc.gpsimd.dma_start(out=r32(xT_sb[:, 2]), in_=xrv[:, 2])
            nc.gpsimd.dma_start(out=r32(xT_sb[:, 3]), in_=xrv[:, 3])
            nc.gpsimd.dma_start(out=r32(wk_sb), in_=wkr.ap())
            nc.gpsimd.dma_start(out=cos_sb, in_=cos4.ap())
            nc.gpsimd.dma_start(out=sin_sb, in_=sin4.ap())
            nc.gpsimd.dma_start(out=r32(wv_sb), in_=wvr.ap())
            nc.gpsimd.dma_start(out=tri_sb, in_=tri.ap())
            nc.gpsimd.dma_start(out=r32(sel_sb), in_=sel.ap())
            nc.gpsimd.dma_start(out=bo_sb, in_=bor)
            nc.gpsimd.dma_start(out=r32(wo_sb), in_=wor.ap())

            ps1 = ph1.enter_context(tc.tile_pool(name="ps1", bufs=2, space="PSUM"))
            tmp_pool = ph1.enter_context(tc.tile_pool(name="tmp", bufs=4))

            def rope_tail(Rc, n):
                # swap 32-row halves via PE perm matmul; sign baked in sin_sb
                sw_ps = ps1.tile([128, TQ], F32, name="sw_ps")
                nc.tensor.matmul(
                    out=sw_ps, lhsT=r32(perm_sb), rhs=r32(Rc), start=True, stop=True
                )
                tmpS = tmp_pool.tile([128, TQ], F32, name="tmpS")
                tmpC = tmp_pool.tile([128, TQ], F32, name="tmpC")
                nc.vector.tensor_mul(tmpS, sw_ps, sin_sb[:, n * TQ : (n + 1) * TQ])
                nc.vector.tensor_mul(tmpC, Rc, cos_sb[:, n * TQ : (n + 1) * TQ])
                nc.vector.tensor_add(r32(Rc), tmpC, tmpS)

            pend = None
            for p in range(NPAIRS):
                for w_sb, b_sb, R in (
                    (wq_sb, bq_sb, RQ[p]),
                    (wk_sb, bk_sb, RK[p]),
                ):
                    for n in range(NQ):
                        u_ps = ps1.tile([128, TQ], F32, name="u_ps")
                        for k in range(KT):
                            nc.tensor.matmul(
                                out=u_ps,
                                lhsT=r32(w_sb[:, k, p * 128 : (p + 1) * 128]),
                                rhs=r32(xT_sb[:, n, k, :]),
                                start=(k == 0),
                                stop=(k == KT - 1),
                            )
                        Rc = R[:, n * TQ : (n + 1) * TQ]
                        # evict with bias -> R buffer (pre-rotation values)
                        nc.scalar.activation(
                            out=r32(Rc),
                            in_=u_ps,
                            func=AF.Identity,
                            bias=b_sb[:, p : p + 1],
                        )
                        if pend is not None:
                            rope_tail(*pend)
                        pend = (Rc, n)
            rope_tail(*pend)

            # V tiles [t,d] with ones column per head
            nc.vector.memset(V_sb, 1.0)
            for tt in range(NKT):
                v_ps = ps1.tile([128, DGRP], F32, name="v_ps")
                for k in range(KT):
                    nc.tensor.matmul(
                        out=v_ps,
                        lhsT=r32(
                            xT_sb[:, tt // 4, k, (tt % 4) * TK : (tt % 4 + 1) * TK]
                        ),
                        rhs=r32(wv_sb[:, k, :]),
                        start=(k == 0),
                        stop=(k == KT - 1),
                    )
                nc.vector.tensor_copy(
                    out=r32(V_sb[:, tt, :, 0:HD]),
                    in_=v_ps.rearrange("p (h d) -> p h d", h=HPG),
                )

        # ---------- Phase 2+3: attention, chunked AG + projection ----------
        with ExitStack() as ph2:
            sd_pool = ph2.enter_context(tc.tile_pool(name="sduo", bufs=2, space="PSUM"))
            av_pool = ph2.enter_context(tc.tile_pool(name="av", bufs=1, space="PSUM"))
            bc_pool = ph2.enter_context(tc.tile_pool(name="bc", bufs=1, space="PSUM"))
            ps3 = ph2.enter_context(tc.tile_pool(name="ps3", bufs=1, space="PSUM"))
            pt_pool = ph2.enter_context(tc.tile_pool(name="ptile", bufs=3))
            o_pool = ph2.enter_context(tc.tile_pool(name="osb", bufs=3))
            ya_pool = ph2.enter_context(tc.tile_pool(name="ya", bufs=2))
            ob_pool = ph2.enter_context(tc.tile_pool(name="ob", bufs=2))
            d_pool = ph2.enter_context(tc.tile_pool(name="dsb", bufs=1))
            dq_pool = ph2.enter_context(tc.tile_pool(name="dq", bufs=2))
            rb_pool = ph2.enter_context(tc.tile_pool(name="rb", bufs=2))
            o_all = d_pool.tile([128, NIDX, TQ], F32)
            av = [av_pool.tile([128, TQ], F32, name=f"av{hh}") for hh in range(2)]

            def issue_av(p, qi, ptile, g2, hh):
                last_ti = 4 * qi + 3
                for ji in range(2):
                    ti = 2 * g2 + ji
                    nc.tensor.matmul(
                        out=av[hh][0:65, :],
                        lhsT=r32(V_sb[:, ti, 2 * p + hh, :]),
                        rhs=r32(ptile[:, ji * TQ : (ji + 1) * TQ]),
                        start=(ti == 0),
                        stop=(ti == last_ti),
                    )

            def proj_chunk(qi):
                ya_sb = ya_pool.tile([128, KT, TQ], F32, name="ya_sb")
                for k in range(KT):
                    nc.gpsimd.dma_start(
                        out=r32(ya_sb[:, k, :]),
                        in_=y_all_q[qi][k * 128 : (k + 1) * 128, :],
                    )
                for mb in range(2):
                    o_ps = ps3.tile([128, TQ], F32, name="o_ps")
                    for k in range(KT):
                        nc.tensor.matmul(
                            out=o_ps,
                            lhsT=r32(wo_sb[:, k, mb * 128 : (mb + 1) * 128]),
                            rhs=r32(ya_sb[:, k, :]),
                            start=(k == 0),
                            stop=(k == KT - 1),
                        )
                    ob = ob_pool.tile([128, TQ], F32, name="ob")
                    nc.scalar.activation(
                        out=ob, in_=o_ps, func=AF.Identity, bias=bo_sb[:, mb : mb + 1]
                    )
                    nc.gpsimd.dma_start(
                        out=out.ap()[
                            mb * 128 : (mb + 1) * 128, qi * TQ : (qi + 1) * TQ
                        ],
                        in_=ob,
                    )

            for qi in range(NQ):
                dq = dq_pool.tile([1, 2 * NPAIRS, TQ], F32, name="dq")
                for p in range(NPAIRS):
                    avpend = None
                    for g2 in range(2 * qi + 2):
                        for hh in range(2):
                            s_ps = sd_pool.tile([128, 2 * TQ], F32, name="s_ps")
                            for ji in range(2):
                                ti = 2 * g2 + ji
                                nc.tensor.matmul(
                                    out=s_ps[:, ji * TQ : (ji + 1) * TQ],
                                    lhsT=r32(
                                        RK[p][
                                            64 * hh : 64 * hh + 64,
                                            ti * TK : (ti + 1) * TK,
                                        ]
                                    ),
                                    rhs=r32(
                                        RQ[p][
                                            64 * hh : 64 * hh + 64,
                                            qi * TQ : (qi + 1) * TQ,
                                        ]
                                    ),
                                    start=True,
                                    stop=True,
                                )
                            ptile = pt_pool.tile([128, 2 * TQ], F32, name="ptile")
                            nc.scalar.activation(
                                out=r32(ptile), in_=s_ps, func=AF.Exp, scale=SCALE
                            )
                            if g2 >= 2 * qi:  # diagonal duo: causal masking
                                for ji in range(2):
                                    ti = 2 * g2 + ji
                                    off = TK * (ti - 4 * qi)
                                    col = ji * TQ
                                    if off > 0:
                                        nc.vector.memset(
                                            ptile[:, col : col + off], 0.0
                                        )
                                    blk = ptile[:, col + off : col + off + TK]
                                    nc.vector.tensor_mul(r32(blk), blk, tri_sb)
                            if avpend is not None:
                                issue_av(p, qi, *avpend)
                            avpend = (ptile, g2, hh)
                    issue_av(p, qi, *avpend)

                    # evict O_aug + denominators for this (p, qi)
                    idx = qi * NPAIRS + p
                    nc.vector.tensor_copy(out=o_all[0:64, idx, :], in_=av[0][0:64, :])
                    nc.vector.tensor_copy(
                        out=o_all[64:128, idx, :], in_=av[1][0:64, :]
                    )
                    # engine partition bases must be 32-aligned, so stage
                    # denom rows in free dim of one partition, scatter via DMA
                    nc.vector.tensor_copy(out=dq[0:1, 2 * p, :], in_=av[0][64:65, :])
                    nc.vector.tensor_copy(
                        out=dq[0:1, 2 * p + 1, :], in_=av[1][64:65, :]
                    )
                    # fill the AG-latency window of chunk qi-2 with its proj
                    if p == 0 and qi >= 2:
                        proj_chunk(qi - 2)

                # normalize + emit y chunk qi; 1/d = exp(-ln d) on ACT (DVE
                # recip on a single partition costs ~15us, ACT ~4us; denom>=1)
                nc.scalar.activation(
                    out=dq[0:1, :, :], in_=dq[0:1, :, :], func=AF.Ln
                )
                nc.scalar.activation(
                    out=dq[0:1, :, :], in_=dq[0:1, :, :], func=AF.Exp, scale=-1.0
                )
                rbuf = rb_pool.tile([2 * NPAIRS, TQ], F32, name="rbuf")
                nc.gpsimd.dma_start(out=r32(rbuf), in_=dq[0:1, :, :])
                for p in range(NPAIRS):
                    idx = qi * NPAIRS + p
                    bc_ps = bc_pool.tile([128, TQ], F32, name="bc_ps")
                    nc.tensor.matmul(
                        out=bc_ps,
                        lhsT=r32(sel_sb[:, p, :]),
                        rhs=r32(rbuf),
                        start=True,
                        stop=True,
                    )
                    yt2 = o_pool.tile([128, TQ], F32, name="yt2")
                    nc.vector.tensor_mul(yt2, o_all[:, idx, :], bc_ps)
                    nc.gpsimd.dma_start(
                        out=y_loc_q[qi][128 * p : 128 * (p + 1), :], in_=yt2
                    )
                nc.gpsimd.collective_compute(
                    "AllGather",
                    ALU.bypass,
                    ins=[y_loc_q[qi].opt()],
                    outs=[y_all_q[qi].opt()],
                    replica_groups=[[0, 1, 2, 3], [4, 5, 6, 7]],
                )
            proj_chunk(NQ - 2)
            proj_chunk(NQ - 1)
    nc.finalize()
    return nc


_NC = None


def _get_nc():
    global _NC
    if _NC is None:
        _NC = build_nc()
    return _NC


def _relay(w):
    # [KT*128, M] -> [128, KT*M] so each partition's DMA line is contiguous
    kt, m = w.shape[0] // 128, w.shape[1]
    return np.ascontiguousarray(
        w.reshape(kt, 128, m).transpose(1, 0, 2).reshape(128, kt * m)
    )


def _relay_x(xb):
    # x^T [C, T] -> [128, NQ*KT*TQ] n-major so each 512-col block is one
    # contiguous-per-partition DMA
    xt = xb.T.reshape(KT, 128, NQ, TQ)
    return np.ascontiguousarray(xt.transpose(1, 2, 0, 3).reshape(128, NQ * KT * TQ))


def _in_maps(x, freqs_cos, freqs_sin, Wqkv, bqkv, Wproj, bproj):
    x = np.asarray(x, np.float32)
    Wqkv = np.asarray(Wqkv, np.float32)
    bqkv = np.asarray(bqkv, np.float32)
    Wproj = np.asarray(Wproj, np.float32)
    bproj = np.asarray(bproj, np.float32)
    cos4 = np.ascontiguousarray(np.tile(np.asarray(freqs_cos, np.float32).T, (4, 1)))
    sinT = np.asarray(freqs_sin, np.float32).T  # [32, T]
    sin4 = np.ascontiguousarray(np.tile(np.concatenate([-sinT, sinT], axis=0), (2, 1)))
    perm = np.zeros((128, 128), np.float32)
    for j in range(128):
        i = j + 32 if (j % 64) < 32 else j - 32
        perm[i, j] = 1.0
    tri = np.triu(np.ones((TK, TK), np.float32))
    sel = np.zeros((2 * NPAIRS, NPAIRS, 128), np.float32)
    for p in range(NPAIRS):
        sel[2 * p, p, 0:64] = 1.0
        sel[2 * p + 1, p, 64:128] = 1.0
    bproj_eff = bproj + bqkv[2 * C : 3 * C] @ Wproj
    maps = []
    for r in range(NCORES):
        b, g = r // GROUPS, r % GROUPS
        sl = slice(DGRP * g, DGRP * (g + 1))
        maps.append(
            {
                "xr": _relay_x(x[b]),
                "wqr": _relay(Wqkv[:, 0 * C :][:, sl]),
                "wkr": _relay(Wqkv[:, 1 * C :][:, sl]),
                "wvr": _relay(Wqkv[:, 2 * C :][:, sl]),
                "wor": _relay(Wproj[:, sl]),
                "cos4": cos4,
                "sin4": sin4,
                "perm": perm,
                "tri": tri,
                "sel": sel,
                "bq": np.ascontiguousarray(bqkv[0 * C : 1 * C][sl]).reshape(DGRP, 1),
                "bk": np.ascontiguousarray(bqkv[1 * C : 2 * C][sl]).reshape(DGRP, 1),
                "bo": np.ascontiguousarray(bproj_eff[sl]).reshape(DGRP, 1),
            }
        )
    return maps


def _assemble(results):
    y = np.empty((B, T, C), np.float32)
    for b in range(B):
        cat = np.concatenate(
            [np.asarray(results[GROUPS * b + g]["out"]) for g in range(GROUPS)], axis=0
        )
        y[b] = cat.T
    return y


def kernel(**inputs):
    nc = _get_nc()
    res = run_bass_kernel_spmd(nc, _in_maps(**inputs), core_ids=list(range(NCORES)))
    return _assemble(res.results)


def kernel_traced(**inputs):
    import tempfile

    nc = _get_nc()
    tmpdir = tempfile.mkdtemp(prefix="mha_trace_")
    res = run_bass_kernel_spmd(
        nc,
        _in_maps(**inputs),
        core_ids=list(range(NCORES)),
        trace=True,
        trace_cores=list(range(NCORES)),
        tmpdir=tmpdir,
    )
    return _assemble(res.results), res.exec_time_ns, tmpdir



# revision 8
# speedup vs baseline: 1.7019x; 1.7019x over previous
"""MHA kernel for trn2: 8 cores = 2 (batch DP) x 4 (head TP, 4 heads/core).

v2 layout strategy (no collectives; host reduces partial projections):
  - x^T [C, T] per batch (host-transposed, partition-relayout), DMA'd in
    fine-grained (n, k-half) blocks ordered to match PE consumption
  - warmup dummy matmuls at t=0 keep the PE HAM clock warm during the
    initial DMA wait
  - Q^T/K^T computed as [d, t] via lhsT=W-slice, rhs=x^T; RoPE via PE
    half-swap permutation matmul + signed sin table
  - V computed as [t, d] tiles (plus ones column for softmax denom);
    tt 0..7 in phase 1, tt 8..15 issued as PE filler inside the small
    qi=0 attention chunk
  - S^T duos [tk=128, 2*512]; 2-head packing (K=64, base partitions
    0/64); causal block skipping; exp per duo on ACT; AV one duo behind
    S so the PE never waits on ACT
  - AV accumulates O_aug^T [65, tq] per head; row 64 = softmax denom
  - normalize per (qi, p): denom rows -> dq -> DMA scatter -> DVE
    reciprocal_approx_fast -> PE broadcast matmul -> DVE mul
  - projection: each core computes a FULL-WIDTH partial out [C, T]
    contracting only its local 256 y-dims (8 o-tiles x K=256 per
    512-col chunk), evicted via ACT and DMA'd per o-tile; proj(qi) is
    issued inside attention(qi+1) to hide the normalize latency
Host reassembles: sum the 4 group partials per batch + bias, transpose.
"""

import sys

sys.path.insert(0, "/opt/trn_rl_repo")

from contextlib import ExitStack  # noqa: E402

import numpy as np  # noqa: E402

import concourse.bacc as bacc  # noqa: E402
import concourse.bass as bass  # noqa: E402
import concourse.tile as tile  # noqa: E402
from concourse import mybir  # noqa: E402
from concourse.bass_utils import run_bass_kernel_spmd  # noqa: E402

B, T, C, H = 2, 2048, 1024, 16
HD, HD2 = 64, 32
NCORES, GROUPS, HPG, NPAIRS = 8, 4, 4, 2
TK, TQ = 128, 512
NQ = T // TQ  # 4 q-chunks
NKT = T // TK  # 16 tk tiles
KT = C // 128  # 8 contraction tiles
DGRP = 256  # head dims per core (4 heads * 64)
NOT = C // 128  # 8 output o-tiles (full width)
NWARM = 5

F32 = mybir.dt.float32
F32R = mybir.dt.float32r
AF = mybir.ActivationFunctionType
SCALE = 1.0 / 8.0  # 1/sqrt(HD)


def r32(ap):
    return ap.bitcast(F32R)


def build_nc():
    nc = bacc.Bacc(target_bir_lowering=False)

    xr = nc.dram_tensor("xr", [128, KT * T], F32, kind="ExternalInput")
    wqr = nc.dram_tensor("wqr", [128, KT * DGRP], F32, kind="ExternalInput")
    wkr = nc.dram_tensor("wkr", [128, KT * DGRP], F32, kind="ExternalInput")
    wvr = nc.dram_tensor("wvr", [128, KT * DGRP], F32, kind="ExternalInput")
    wor = nc.dram_tensor("wor", [128, 2 * C], F32, kind="ExternalInput")
    cos4 = nc.dram_tensor("cos4", [128, T], F32, kind="ExternalInput")
    sin4 = nc.dram_tensor("sin4", [128, T], F32, kind="ExternalInput")
    perm = nc.dram_tensor("perm", [128, 128], F32, kind="ExternalInput")
    tri = nc.dram_tensor("tri", [TK, TK], F32, kind="ExternalInput")
    bq = nc.dram_tensor("bq", [DGRP, 1], F32, kind="ExternalInput")
    bk = nc.dram_tensor("bk", [DGRP, 1], F32, kind="ExternalInput")
    sel = nc.dram_tensor("sel", [2, 128], F32, kind="ExternalInput")
    out = nc.dram_tensor("out", [C, T], F32, kind="ExternalOutput")

    with tile.TileContext(nc) as tc, ExitStack() as top:
        consts = top.enter_context(tc.tile_pool(name="consts", bufs=1))
        cos_sb = consts.tile([128, T], F32)
        sin_sb = consts.tile([128, T], F32)
        perm_sb = consts.tile([128, 128], F32)
        tri_sb = consts.tile([TK, TK], F32)
        bq_sb = consts.tile([128, NPAIRS], F32)
        bk_sb = consts.tile([128, NPAIRS], F32)
        sel_sb = consts.tile([2, 128], F32)
        bqr = bq.ap().rearrange("(p c) one -> c (p one)", c=128)
        bkr = bk.ap().rearrange("(p c) one -> c (p one)", c=128)

        wo_pool = top.enter_context(tc.tile_pool(name="wo", bufs=1))
        wo_sb = wo_pool.tile([128, 2, C], F32)

        rqk_pool = top.enter_context(tc.tile_pool(name="rqk", bufs=1))
        # RQ/RK per pair: [128, T]; rows = (u1 h0, u2 h0, u1 h1, u2 h1) x 32
        RQ = [rqk_pool.tile([128, T], F32, name=f"RQ{p}") for p in range(NPAIRS)]
        RK = [rqk_pool.tile([128, T], F32, name=f"RK{p}") for p in range(NPAIRS)]
        v_pool = top.enter_context(tc.tile_pool(name="vsb", bufs=1))
        V_sb = v_pool.tile([128, NKT, HPG, HD + 1], F32)

        xv = ExitStack()
        xt_pool = xv.enter_context(tc.tile_pool(name="xt", bufs=1, side="right"))
        xT_sb = xt_pool.tile([128, NQ, KT, TQ], F32)
        wv_pool = xv.enter_context(tc.tile_pool(name="wv", bufs=1, side="right"))
        wv_sb = wv_pool.tile([128, KT, DGRP], F32)

        # ---------------- Phase 1: QKV projection + RoPE ----------------
        ph1 = ExitStack()
        wqk_pool = ph1.enter_context(tc.tile_pool(name="wqk", bufs=1))
        wq_sb = wqk_pool.tile([128, KT, DGRP], F32)
        wk_sb = wqk_pool.tile([128, KT, DGRP], F32)

        # DMA issue order = completion order: feed the PE's phase-1
        # consumption sequence (Q p0 needs wq p0 + x n-blocks first)
        xrv = xr.ap().rearrange("p (n k t) -> p n k t", n=NQ, k=KT)
        wqv = wqr.ap().rearrange("p (k d) -> p k d", k=KT)
        wkv = wkr.ap().rearrange("p (k d) -> p k d", k=KT)
        dma = nc.gpsimd.dma_start
        dma(out=r32(wq_sb[:, :, 0:128]), in_=wqv[:, :, 0:128])
        dma(out=r32(xT_sb[:, 0, 0:4]), in_=xrv[:, 0, 0:4])
        dma(out=r32(xT_sb[:, 0, 4:8]), in_=xrv[:, 0, 4:8])
        dma(out=r32(perm_sb), in_=perm.ap())
        dma(out=bq_sb, in_=bqr)
        dma(out=cos_sb[:, 0:TQ], in_=cos4.ap()[:, 0:TQ])
        dma(out=sin_sb[:, 0:TQ], in_=sin4.ap()[:, 0:TQ])
        dma(out=r32(xT_sb[:, 1, 0:4]), in_=xrv[:, 1, 0:4])
        dma(out=r32(xT_sb[:, 1, 4:8]), in_=xrv[:, 1, 4:8])
        dma(out=cos_sb[:, TQ : 2 * TQ], in_=cos4.ap()[:, TQ : 2 * TQ])
        dma(out=sin_sb[:, TQ : 2 * TQ], in_=sin4.ap()[:, TQ : 2 * TQ])
        dma(out=r32(xT_sb[:, 2, 0:4]), in_=xrv[:, 2, 0:4])
        dma(out=r32(xT_sb[:, 2, 4:8]), in_=xrv[:, 2, 4:8])
        dma(out=cos_sb[:, 2 * TQ : 3 * TQ], in_=cos4.ap()[:, 2 * TQ : 3 * TQ])
        dma(out=sin_sb[:, 2 * TQ : 3 * TQ], in_=sin4.ap()[:, 2 * TQ : 3 * TQ])
        dma(out=r32(xT_sb[:, 3, 0:4]), in_=xrv[:, 3, 0:4])
        dma(out=r32(xT_sb[:, 3, 4:8]), in_=xrv[:, 3, 4:8])
        dma(out=cos_sb[:, 3 * TQ :], in_=cos4.ap()[:, 3 * TQ :])
        dma(out=sin_sb[:, 3 * TQ :], in_=sin4.ap()[:, 3 * TQ :])
        dma(out=r32(wk_sb[:, :, 0:128]), in_=wkv[:, :, 0:128])
        dma(out=bk_sb, in_=bkr)
        dma(out=r32(wq_sb[:, :, 128:256]), in_=wqv[:, :, 128:256])
        dma(out=r32(wk_sb[:, :, 128:256]), in_=wkv[:, :, 128:256])
        dma(out=r32(wv_sb), in_=wvr.ap())
        dma(out=tri_sb, in_=tri.ap())
        dma(out=r32(sel_sb), in_=sel.ap())
        dma(out=r32(wo_sb), in_=wor.ap())

        ps1 = ph1.enter_context(tc.tile_pool(name="ps1", bufs=2, space="PSUM"))
        tmp_pool = ph1.enter_context(tc.tile_pool(name="tmp", bufs=4))

        # warm the PE HAM clock gate during the initial DMA wait
        warm_pool = ph1.enter_context(tc.tile_pool(name="warm", bufs=1))
        warm_sb = warm_pool.tile([128, TQ], F32)
        wps_pool = ph1.enter_context(tc.tile_pool(name="wps", bufs=1, space="PSUM"))
        warm_ps = wps_pool.tile([128, TQ], F32)
        nc.vector.memset(warm_sb, 0.0)
        for _ in range(NWARM):
            nc.tensor.matmul(
                out=warm_ps,
                lhsT=warm_sb[:, 0:128],
                rhs=warm_sb,
                start=True,
                stop=True,
            )

        def rope_tail(Rc, n):
            # swap 32-row halves via PE perm matmul; sign baked in sin_sb
            sw_ps = ps1.tile([128, TQ], F32, name="sw_ps")
            nc.tensor.matmul(
                out=sw_ps, lhsT=r32(perm_sb), rhs=r32(Rc), start=True, stop=True
            )
            tmpS = tmp_pool.tile([128, TQ], F32, name="tmpS")
            tmpC = tmp_pool.tile([128, TQ], F32, name="tmpC")
            nc.vector.tensor_mul(tmpS, sw_ps, sin_sb[:, n * TQ : (n + 1) * TQ])
            nc.vector.tensor_mul(tmpC, Rc, cos_sb[:, n * TQ : (n + 1) * TQ])
            nc.vector.tensor_add(r32(Rc), tmpC, tmpS)

        pend = None
        for p in range(NPAIRS):
            for w_sb, b_sb, R in (
                (wq_sb, bq_sb, RQ[p]),
                (wk_sb, bk_sb, RK[p]),
            ):
                for n in range(NQ):
                    u_ps = ps1.tile([128, TQ], F32, name="u_ps")
                    for k in range(KT):
                        nc.tensor.matmul(
                            out=u_ps,
                            lhsT=r32(w_sb[:, k, p * 128 : (p + 1) * 128]),
                            rhs=r32(xT_sb[:, n, k, :]),
                            start=(k == 0),
                            stop=(k == KT - 1),
                        )
                    Rc = R[:, n * TQ : (n + 1) * TQ]
                    # evict with bias -> R buffer (pre-rotation values)
                    nc.scalar.activation(
                        out=r32(Rc),
                        in_=u_ps,
                        func=AF.Identity,
                        bias=b_sb[:, p : p + 1],
                    )
                    if pend is not None:
                        rope_tail(*pend)
                    pend = (Rc, n)
        rope_tail(*pend)

        # V tiles [t,d] with ones column per head; tt 0..7 here, rest as
        # filler inside the qi=0 attention chunk
        nc.vector.memset(V_sb, 1.0)

        def v_tile(tt, ps_pool, width, psname="v_ps"):
            v_ps = ps_pool.tile([128, width], F32, name=psname)
            for k in range(KT):
                nc.tensor.matmul(
                    out=v_ps[:, 0:DGRP],
                    lhsT=r32(
                        xT_sb[:, tt // 4, k, (tt % 4) * TK : (tt % 4 + 1) * TK]
                    ),
                    rhs=r32(wv_sb[:, k, :]),
                    start=(k == 0),
                    stop=(k == KT - 1),
                )
            nc.vector.tensor_copy(
                out=r32(V_sb[:, tt, :, 0:HD]),
                in_=v_ps[:, 0:DGRP].rearrange("p (h d) -> p h d", h=HPG),
            )

        for tt in range(8):
            v_tile(tt, ps1, DGRP)
        ph1.close()

        # ---------- Phase 2: attention + interleaved partial projection ----
        ph2 = ExitStack()
        sd_pool = ph2.enter_context(tc.tile_pool(name="sduo", bufs=2, space="PSUM"))
        av_pool = ph2.enter_context(tc.tile_pool(name="av", bufs=1, space="PSUM"))
        pp_pool = ph2.enter_context(tc.tile_pool(name="pp", bufs=2, space="PSUM"))
        pt_pool = ph2.enter_context(tc.tile_pool(name="ptile", bufs=3))
        ost_pool = ph2.enter_context(tc.tile_pool(name="ost", bufs=3))
        yt_pool = ph2.enter_context(tc.tile_pool(name="yt", bufs=4))
        ob_pool = ph2.enter_context(tc.tile_pool(name="ob", bufs=3))
        dq_pool = ph2.enter_context(tc.tile_pool(name="dq", bufs=2))
        rb_pool = ph2.enter_context(tc.tile_pool(name="rb", bufs=2))
        rr_pool = ph2.enter_context(tc.tile_pool(name="rr", bufs=2))
        av = [av_pool.tile([128, TQ], F32, name=f"av{hh}") for hh in range(2)]
        OST = {}  # (qi, p) -> unnormalized O tile
        RRT = {}  # (qi, p) -> reciprocal denominator [2, TQ]
        YT = {}  # (qi, p) -> normalized y tile

        def issue_av(p, qi, ptile, g2, hh):
            last_ti = 4 * qi + 3
            for ji in range(2):
                ti = 2 * g2 + ji
                nc.tensor.matmul(
                    out=av[hh][0:65, :],
                    lhsT=r32(V_sb[:, ti, 2 * p + hh, :]),
                    rhs=r32(ptile[:, ji * TQ : (ji + 1) * TQ]),
                    start=(ti == 0),
                    stop=(ti == last_ti),
                )

        def attn_block(qi, p):
            avpend = None
            for g2 in range(2 * qi + 2):
                for hh in range(2):
                    s_ps = sd_pool.tile([128, 2 * TQ], F32, name="s_ps")
                    for ji in range(2):
                        ti = 2 * g2 + ji
                        nc.tensor.matmul(
                            out=s_ps[:, ji * TQ : (ji + 1) * TQ],
                            lhsT=r32(
                                RK[p][
                                    64 * hh : 64 * hh + 64,
                                    ti * TK : (ti + 1) * TK,
                                ]
                            ),
                            rhs=r32(
                                RQ[p][
                                    64 * hh : 64 * hh + 64,
                                    qi * TQ : (qi + 1) * TQ,
                                ]
                            ),
                            start=True,
                            stop=True,
                        )
                    ptile = pt_pool.tile([128, 2 * TQ], F32, name="ptile")
                    nc.scalar.activation(
                        out=r32(ptile), in_=s_ps, func=AF.Exp, scale=SCALE
                    )
                    if g2 >= 2 * qi:  # diagonal duo: causal masking
                        for ji in range(2):
                            ti = 2 * g2 + ji
                            off = TK * (ti - 4 * qi)
                            col = ji * TQ
                            if off > 0:
                                nc.vector.memset(ptile[:, col : col + off], 0.0)
                            blk = ptile[:, col + off : col + off + TK]
                            nc.vector.tensor_mul(r32(blk), blk, tri_sb)
                    if avpend is not None:
                        issue_av(p, qi, *avpend)
                    avpend = (ptile, g2, hh)
            issue_av(p, qi, *avpend)

            # evict O + denominators; start the reciprocal chain
            o_st = ost_pool.tile([128, TQ], F32, name="o_st")
            nc.vector.tensor_copy(out=o_st[0:64, :], in_=av[0][0:64, :])
            nc.vector.tensor_copy(out=o_st[64:128, :], in_=av[1][0:64, :])
            dq = dq_pool.tile([1, 2, TQ], F32, name="dq")
            nc.vector.tensor_copy(out=dq[0:1, 0, :], in_=av[0][64:65, :])
            nc.vector.tensor_copy(out=dq[0:1, 1, :], in_=av[1][64:65, :])
            rd = rr_pool.tile([1, 2, TQ], F32, name="rd")
            nc.vector.reciprocal_approx_fast(out=rd[0:1, :, :], in_=dq[0:1, :, :])
            rb = rb_pool.tile([2, TQ], F32, name="rb")
            nc.gpsimd.dma_start(out=r32(rb), in_=rd[0:1, :, :])
            OST[(qi, p)] = o_st
            RRT[(qi, p)] = rb

        def bc_block(qi, p):
            # broadcast 1/denom across partitions via PE, normalize y tile
            bc_ps = pp_pool.tile([128, TQ], F32, name="pp_t")
            nc.tensor.matmul(
                out=bc_ps,
                lhsT=r32(sel_sb),
                rhs=r32(RRT[(qi, p)]),
                start=True,
                stop=True,
            )
            yt = yt_pool.tile([128, TQ], F32, name="yt")
            nc.vector.tensor_mul(r32(yt), OST[(qi, p)], bc_ps)
            YT[(qi, p)] = yt

        def proj_chunk(qi):
            # full-width partial projection of chunk qi (K=256 local dims)
            for ot in range(NOT):
                o_ps = pp_pool.tile([128, TQ], F32, name="pp_t")
                for j in range(2):
                    nc.tensor.matmul(
                        out=o_ps,
                        lhsT=r32(wo_sb[:, j, ot * 128 : (ot + 1) * 128]),
                        rhs=r32(YT[(qi, j)]),
                        start=(j == 0),
                        stop=(j == 1),
                    )
                ob = ob_pool.tile([128, TQ], F32, name="ob")
                nc.scalar.activation(out=ob, in_=o_ps, func=AF.Identity)
                nc.sync.dma_start(
                    out=out.ap()[ot * 128 : (ot + 1) * 128, qi * TQ : (qi + 1) * TQ],
                    in_=ob,
                )

        attn_block(0, 0)
        for tt in range(8, 12):
            v_tile(tt, pp_pool, TQ, "pp_t")
        bc_block(0, 0)
        attn_block(0, 1)
        for tt in range(12, 16):
            v_tile(tt, pp_pool, TQ, "pp_t")
        xv.close()
        for qi in range(1, NQ):
            attn_block(qi, 0)
            bc_block(qi - 1, 1)
            proj_chunk(qi - 1)
            bc_block(qi, 0)
            attn_block(qi, 1)
        bc_block(NQ - 1, 1)
        proj_chunk(NQ - 1)
        ph2.close()
    nc.finalize()
    return nc


_NC = None


def _get_nc():
    global _NC
    if _NC is None:
        _NC = build_nc()
    return _NC


def _relay(w):
    # [KT*128, M] -> [128, KT*M] so each partition's DMA line is contiguous
    kt, m = w.shape[0] // 128, w.shape[1]
    return np.ascontiguousarray(
        w.reshape(kt, 128, m).transpose(1, 0, 2).reshape(128, kt * m)
    )


def _relay_x(xb):
    # x^T [C, T] -> [128, NQ*KT*TQ] n-major so each 512-col block is one
    # contiguous-per-partition DMA
    xt = xb.T.reshape(KT, 128, NQ, TQ)
    return np.ascontiguousarray(xt.transpose(1, 2, 0, 3).reshape(128, NQ * KT * TQ))


def _prep(x, freqs_cos, freqs_sin, Wqkv, bqkv, Wproj, bproj):
    x = np.asarray(x, np.float32)
    Wqkv = np.asarray(Wqkv, np.float32)
    bqkv = np.asarray(bqkv, np.float32)
    Wproj = np.asarray(Wproj, np.float32)
    bproj = np.asarray(bproj, np.float32)
    cos4 = np.ascontiguousarray(np.tile(np.asarray(freqs_cos, np.float32).T, (4, 1)))
    sinT = np.asarray(freqs_sin, np.float32).T  # [32, T]
    sin4 = np.ascontiguousarray(np.tile(np.concatenate([-sinT, sinT], axis=0), (2, 1)))
    perm = np.zeros((128, 128), np.float32)
    for j in range(128):
        i = j + 32 if (j % 64) < 32 else j - 32
        perm[i, j] = 1.0
    tri = np.triu(np.ones((TK, TK), np.float32))
    sel = np.zeros((2, 128), np.float32)
    sel[0, 0:64] = 1.0
    sel[1, 64:128] = 1.0
    bproj_eff = bproj + bqkv[2 * C : 3 * C] @ Wproj
    maps = []
    for r in range(NCORES):
        b, g = r // GROUPS, r % GROUPS
        sl = slice(DGRP * g, DGRP * (g + 1))
        maps.append(
            {
                "xr": _relay_x(x[b]),
                "wqr": _relay(Wqkv[:, 0 * C :][:, sl]),
                "wkr": _relay(Wqkv[:, 1 * C :][:, sl]),
                "wvr": _relay(Wqkv[:, 2 * C :][:, sl]),
                "wor": _relay(Wproj[sl, :]),
                "cos4": cos4,
                "sin4": sin4,
                "perm": perm,
                "tri": tri,
                "sel": sel,
                "bq": np.ascontiguousarray(bqkv[0 * C : 1 * C][sl]).reshape(DGRP, 1),
                "bk": np.ascontiguousarray(bqkv[1 * C : 2 * C][sl]).reshape(DGRP, 1),
            }
        )
    return maps, bproj_eff


def _assemble(results, bias):
    y = np.empty((B, T, C), np.float32)
    for b in range(B):
        acc = np.array(results[GROUPS * b]["out"], np.float32, copy=True)
        for g in range(1, GROUPS):
            acc += np.asarray(results[GROUPS * b + g]["out"])
        y[b] = (acc + bias[:, None]).T
    return y


def kernel(**inputs):
    nc = _get_nc()
    maps, bias = _prep(**inputs)
    res = run_bass_kernel_spmd(nc, maps, core_ids=list(range(NCORES)))
    return _assemble(res.results, bias)


def kernel_traced(**inputs):
    import tempfile

    nc = _get_nc()
    maps, bias = _prep(**inputs)
    tmpdir = tempfile.mkdtemp(prefix="mha_trace_")
    res = run_bass_kernel_spmd(
        nc,
        maps,
        core_ids=list(range(NCORES)),
        trace=True,
        trace_cores=list(range(NCORES)),
        tmpdir=tmpdir,
    )
    return _assemble(res.results, bias), res.exec_time_ns, tmpdir
